# revision 6
# baseline (speedup 1.0000x reference)
"""Pairwise L2 distance kernel: x [4096,768], anchors [100,64,768] -> [4096,100,64].

Distributed over 8 TRN2 NeuronCores as a 2x4 grid: batch (4096) split in 2,
anchor index (6400) split in 4.  Each core computes a [2048,1600] output block
as sqrt(x2[b] + a2[j] - 2*x@A^T).

The x@A^T matmul runs in fp8e4m3 with DoubleRow (K=256 per pass, fp32 PSUM
accumulate); norms are computed on device (x2 from a bf16 copy of x via
ACT Square+accum, a2 via DVE square + all-(-0.5) ones-matmul broadcast).
Host does layout transforms only (transpose, dtype cast, partition packing).
"""

import sys

import numpy as np

for _p in ("/opt/trn_rl_repo", "/root/.axon_site/_ro/trn_rl_repo"):
    if _p not in sys.path:
        sys.path.append(_p)

import ml_dtypes

import concourse.bass as bass
import concourse.tile as tile
from concourse import bacc, mybir
from concourse.bass import ts
from concourse.bass_utils import run_bass_kernel_spmd

B, C, A, E = 4096, 100, 64, 768
J = C * A                 # 6400 flattened anchors
RB, RJ = 2, 4             # batch groups x anchor groups = 8 cores
MB = B // RB              # 2048 batch rows per core
NJ = J // RJ              # 1600 anchor cols per core
KT = E // 128             # 6 contraction tiles of 128
MT = MB // 128            # 16 m-tiles per core
XO_Q = 4                  # xo arrives in 4 DMA slices
N_CHUNKS = [(0, 512), (512, 512), (1024, 512), (1536, 64)]

FP8 = mybir.dt.float8e4
BF16 = mybir.dt.bfloat16
F32 = mybir.dt.float32
NP_FP8 = ml_dtypes.float8_e4m3
NP_BF16 = ml_dtypes.bfloat16


def pack_rows(a2d: np.ndarray) -> np.ndarray:
    """[n*128, F] -> [128, n*F]: row r=k*128+p lands at partition p, block k.
    Makes each SBUF partition's data contiguous in DRAM (one fat DMA
    descriptor per partition instead of one per 128-row block)."""
    n = a2d.shape[0] // 128
    return np.ascontiguousarray(
        a2d.reshape(n, 128, a2d.shape[1]).transpose(1, 0, 2).reshape(128, -1)
    )


def build_graph() -> bass.Bass:
    mt_q = MT // XO_Q
    nc = bacc.Bacc(None, target_bir_lowering=False, debug=False, num_devices=8)
    at_ext = nc.declare_dram_parameter("at", [128, KT * NJ], FP8, isOutput=False)
    xt_ext = nc.declare_dram_parameter("xt", [128, KT * MB], FP8, isOutput=False)
    xo_ext = nc.declare_dram_parameter("xo", [128, MT * E], BF16, isOutput=False)
    out_ext = nc.declare_dram_parameter("out", [MB, NJ], BF16, isOutput=True)

    with tile.TileContext(nc) as tc:
        with (
            tc.tile_pool(name="big", bufs=1) as big,
            tc.tile_pool(name="kt", bufs=KT) as ktp,
            tc.tile_pool(name="xop", bufs=XO_Q) as xop,
            tc.tile_pool(name="work", bufs=4) as work,
            tc.tile_pool(name="outs", bufs=8) as outs,
            tc.tile_pool(name="psum", bufs=8, space=bass.MemorySpace.PSUM) as psp,
        ):
            at_s = big.tile([128, KT, NJ], FP8)
            nc.sync.dma_start(
                out=at_s, in_=at_ext[:].rearrange("p (k n) -> p k n", k=KT)
            )
            xt_s = big.tile([128, KT, MB], FP8)
            nc.sync.dma_start(
                out=xt_s, in_=xt_ext[:].rearrange("p (k b) -> p k b", k=KT)
            )
            xo_r = xo_ext[:].rearrange("p (q e) -> p q e", q=XO_Q)
            xo_s = []
            for q in range(XO_Q):
                o_t = xop.tile([128, mt_q * E], BF16, tag="xo", name=f"xo{q}")
                nc.sync.dma_start(out=o_t, in_=xo_r[:, q, :])
                xo_s.append(o_t)

            # sq_at = at*at in bf16 (feeds the a2 broadcast matmul).
            sq_at = []
            for k in range(KT):
                s_t = ktp.tile([128, NJ], BF16, tag="sqat", name=f"sqat{k}")
                nc.vector.tensor_mul(s_t, at_s[:, k, :], at_s[:, k, :])
                sq_at.append(s_t)

            neg_half = big.tile([128, 128], BF16)
            nc.vector.memset(neg_half, -0.5)
            a2b = big.tile([128, NJ], F32)  # -0.5*a2[j], same on every partition

            def emit_a2_setup():
                for n0, w in N_CHUNKS:
                    ps = psp.tile([128, 512], F32, tag="ps", name=f"psa2_{n0}")
                    for k in range(KT):
                        nc.tensor.matmul(
                            ps[:, :w], neg_half, sq_at[k][:, n0 : n0 + w],
                            start=(k == 0), stop=(k == KT - 1),
                        )
                    nc.vector.tensor_copy(a2b[:, n0 : n0 + w], ps[:, :w])

            for m in range(MT):
                pts = []
                for ci, (n0, w) in enumerate(N_CHUNKS):
                    pts.append(psp.tile([128, 512], F32, tag="ps", name=f"ps{m}_{ci}"))
                for k2 in range(KT // 2):
                    lhsT = xt_s[:, 2 * k2 : 2 * k2 + 2, ts(m, 128)]
                    for ci, (n0, w) in enumerate(N_CHUNKS):
                        nc.tensor.matmul(
                            pts[ci][:, :w],
                            lhsT,
                            at_s[:, 2 * k2 : 2 * k2 + 2, n0 : n0 + w],
                            start=(k2 == 0), stop=(k2 == KT // 2 - 1),
                            perf_mode=mybir.MatmulPerfMode.DoubleRow,
                        )
                if m == 0:
                    # Traced after m0's matmuls: PE reaches these once at has
                    # landed; the result is ready for m0's epilogue.
                    emit_a2_setup()

                sq_x = work.tile([128, E], BF16, tag="sqx")
                x2 = work.tile([128, 1], F32, tag="x2")
                nc.scalar.activation(
                    sq_x, xo_s[m // mt_q][:, (m % mt_q) * E : (m % mt_q + 1) * E],
                    mybir.ActivationFunctionType.Square, accum_out=x2,
                )

                for ci, (n0, w) in enumerate(N_CHUNKS):
                    t = work.tile([128, 512], F32, tag="t")
                    nc.vector.tensor_add(t[:, :w], pts[ci][:, :w], a2b[:, n0 : n0 + w])
                    out_c = outs.tile([128, 512], BF16, tag="out", name=f"out{m}_{ci}")
                    nc.scalar.activation(
                        out_c[:, :w], t[:, :w],
                        mybir.ActivationFunctionType.Sqrt,
                        bias=x2, scale=-2.0,
                    )
                    nc.sync.dma_start(
                        out=out_ext[ts(m, 128), n0 : n0 + w], in_=out_c[:, :w]
                    )

    nc.compile()
    return nc


def make_in_maps(x32: np.ndarray, a32: np.ndarray) -> list[dict[str, np.ndarray]]:
    xt_f8 = x32.T.astype(NP_FP8)           # [E, B]
    xo_bf = x32.astype(NP_BF16)            # [B, E]
    at_f8 = a32.T.astype(NP_FP8)           # [E, J]
    in_maps = []
    for c in range(8):
        g, h = c // RJ, c % RJ
        in_maps.append({
            "at": pack_rows(at_f8[:, h * NJ : (h + 1) * NJ]),
            "xt": pack_rows(xt_f8[:, g * MB : (g + 1) * MB]),
            "xo": pack_rows(xo_bf[g * MB : (g + 1) * MB, :]),
        })
    return in_maps


def kernel(x: np.ndarray, anchors: np.ndarray) -> np.ndarray:
    x32 = np.asarray(x, dtype=np.float32)
    a32 = np.asarray(anchors, dtype=np.float32).reshape(J, E)

    nc = build_graph()
    in_maps = make_in_maps(x32, a32)
    results = run_bass_kernel_spmd(nc, in_maps, core_ids=list(range(8))).results

    out = np.empty((B, J), dtype=np.float32)
    for c in range(8):
        g, h = c // RJ, c % RJ
        out[g * MB : (g + 1) * MB, h * NJ : (h + 1) * NJ] = results[c][
            "out"
        ].astype(np.float32)
    return out.reshape(B, C, A)


# revision 8
# speedup vs baseline: 1.2511x; 1.2511x over previous
"""Pairwise L2 distance kernel: x [4096,768], anchors [100,64,768] -> [4096,100,64].

Distributed over 8 TRN2 NeuronCores as a 2x4 grid: batch (4096) split in 2,
anchor index (6400) split in 4.  Each core computes a [2048,1600] output block
as sqrt(x2[b] + a2[j] - 2*x@A^T).

The x@A^T matmul runs in fp8e4m3 with DoubleRow (K=256 per pass, fp32 PSUM
accumulate); norms are computed on device (x2 from a bf16 copy of x via
ACT Square+accum, a2 via DVE square + all-(-0.5) ones-matmul broadcast).
Host does layout transforms only (transpose, dtype cast, partition packing).
"""

import sys

import numpy as np

for _p in ("/opt/trn_rl_repo", "/root/.axon_site/_ro/trn_rl_repo"):
    if _p not in sys.path:
        sys.path.append(_p)

import ml_dtypes

import concourse.bass as bass
import concourse.tile as tile
from concourse import bacc, mybir
from concourse.bass import ts
from concourse.bass_utils import run_bass_kernel_spmd

B, C, A, E = 4096, 100, 64, 768
J = C * A                 # 6400 flattened anchors
RB, RJ = 2, 4             # batch groups x anchor groups = 8 cores
MB = B // RB              # 2048 batch rows per core
NJ = J // RJ              # 1600 anchor cols per core
KT = E // 128             # 6 contraction tiles of 128
K2 = KT // 2              # 3 DoubleRow k-pair passes
MT = MB // 128            # 16 m-tiles per core
XO_Q = 4                  # xo arrives in 4 DMA slices
N_CHUNKS = [(0, 512), (512, 512), (1024, 512), (1536, 64)]
PSW = 2048                # psum tile width (4 banks), holds all chunks

FP8 = mybir.dt.float8e4
BF16 = mybir.dt.bfloat16
F32 = mybir.dt.float32
NP_FP8 = ml_dtypes.float8_e4m3
NP_BF16 = ml_dtypes.bfloat16


def pack_rows(a2d: np.ndarray) -> np.ndarray:
    """[n*128, F] -> [128, n*F]: row r=k*128+p lands at partition p, block k.
    Makes each SBUF partition's data contiguous in DRAM (one fat DMA
    descriptor per partition instead of one per 128-row block)."""
    n = a2d.shape[0] // 128
    return np.ascontiguousarray(
        a2d.reshape(n, 128, a2d.shape[1]).transpose(1, 0, 2).reshape(128, -1)
    )


def build_graph() -> bass.Bass:
    mt_q = MT // XO_Q
    nc = bacc.Bacc(None, target_bir_lowering=False, debug=False, num_devices=8)
    at_ext = nc.declare_dram_parameter("at", [128, KT * NJ], FP8, isOutput=False)
    xt_ext = nc.declare_dram_parameter("xt", [128, KT * MB], FP8, isOutput=False)
    xo_ext = nc.declare_dram_parameter("xo", [128, MT * E], BF16, isOutput=False)
    out_ext = nc.declare_dram_parameter("out", [MB, NJ], BF16, isOutput=True)

    with tile.TileContext(nc) as tc:
        with (
            tc.tile_pool(name="big", bufs=1) as big,
            tc.tile_pool(name="k2p", bufs=K2) as k2p,
            tc.tile_pool(name="xop", bufs=XO_Q) as xop,
            tc.tile_pool(name="work", bufs=3) as work,
            tc.tile_pool(name="outs", bufs=3) as outs,
            tc.tile_pool(name="psum", bufs=2, space=bass.MemorySpace.PSUM) as psp,
        ):
            # Per-k2-pair loads so the first matmuls start as soon as the
            # first 256 contraction rows have landed.
            at2, xt2 = [], []
            for q in range(K2):
                a_t = k2p.tile([128, 2, NJ], FP8, tag="at", name=f"at{q}")
                nc.sync.dma_start(
                    out=a_t,
                    in_=at_ext[:, 2 * q * NJ : 2 * (q + 1) * NJ].rearrange(
                        "p (k n) -> p k n", k=2
                    ),
                )
                at2.append(a_t)
                x_t = k2p.tile([128, 2, MB], FP8, tag="xt", name=f"xt{q}")
                nc.sync.dma_start(
                    out=x_t,
                    in_=xt_ext[:, 2 * q * MB : 2 * (q + 1) * MB].rearrange(
                        "p (k b) -> p k b", k=2
                    ),
                )
                xt2.append(x_t)
            xo_r = xo_ext[:].rearrange("p (q e) -> p q e", q=XO_Q)
            xo_s = []
            for q in range(XO_Q):
                o_t = xop.tile([128, mt_q * E], BF16, tag="xo", name=f"xo{q}")
                nc.sync.dma_start(out=o_t, in_=xo_r[:, q, :])
                xo_s.append(o_t)

            # sq_at = at*at in bf16 (feeds the a2 broadcast matmul).
            sq2 = []
            for q in range(K2):
                s_t = k2p.tile([128, 2, NJ], BF16, tag="sqat", name=f"sqat{q}")
                nc.vector.tensor_mul(s_t, at2[q], at2[q])
                sq2.append(s_t)

            neg_half = big.tile([128, 128], BF16)
            nc.vector.memset(neg_half, -0.5)
            a2b = big.tile([128, NJ], F32)  # -0.5*a2[j], same on every partition

            def emit_a2_setup():
                ps = psp.tile([128, PSW], F32, tag="ps", name="psa2")
                for n0, w in N_CHUNKS:
                    for k in range(KT):
                        nc.tensor.matmul(
                            ps[:, n0 : n0 + w],
                            neg_half,
                            sq2[k // 2][:, k % 2, n0 : n0 + w],
                            start=(k == 0), stop=(k == KT - 1),
                        )
                nc.vector.tensor_copy(a2b, ps[:, :NJ])

            for m in range(MT):
                pts = psp.tile([128, PSW], F32, tag="ps", name=f"ps{m}")
                for q in range(K2):
                    lhsT = xt2[q][:, :, ts(m, 128)]
                    for n0, w in N_CHUNKS:
                        nc.tensor.matmul(
                            pts[:, n0 : n0 + w],
                            lhsT,
                            at2[q][:, :, n0 : n0 + w],
                            start=(q == 0), stop=(q == K2 - 1),
                            perf_mode=mybir.MatmulPerfMode.DoubleRow,
                        )
                if m == 0:
                    # Traced after m0's matmuls: PE reaches these once at has
                    # landed; the result is ready for m0's epilogue.
                    emit_a2_setup()

                sq_x = work.tile([128, E], BF16, tag="sqx")
                x2 = work.tile([128, 1], F32, tag="x2")
                nc.scalar.activation(
                    sq_x, xo_s[m // mt_q][:, (m % mt_q) * E : (m % mt_q + 1) * E],
                    mybir.ActivationFunctionType.Square, accum_out=x2,
                )

                t = work.tile([128, NJ], F32, tag="t")
                nc.vector.tensor_add(t, pts[:, :NJ], a2b)
                out_t = outs.tile([128, NJ], BF16, tag="out", name=f"out{m}")
                nc.scalar.activation(
                    out_t, t, mybir.ActivationFunctionType.Sqrt,
                    bias=x2, scale=-2.0,
                )
                nc.sync.dma_start(out=out_ext[ts(m, 128), :], in_=out_t)

    nc.compile()
    return nc


def make_in_maps(x32: np.ndarray, a32: np.ndarray) -> list[dict[str, np.ndarray]]:
    xt_f8 = x32.T.astype(NP_FP8)           # [E, B]
    xo_bf = x32.astype(NP_BF16)            # [B, E]
    at_f8 = a32.T.astype(NP_FP8)           # [E, J]
    in_maps = []
    for c in range(8):
        g, h = c // RJ, c % RJ
        in_maps.append({
            "at": pack_rows(at_f8[:, h * NJ : (h + 1) * NJ]),
            "xt": pack_rows(xt_f8[:, g * MB : (g + 1) * MB]),
            "xo": pack_rows(xo_bf[g * MB : (g + 1) * MB, :]),
        })
    return in_maps


def kernel(x: np.ndarray, anchors: np.ndarray) -> np.ndarray:
    x32 = np.asarray(x, dtype=np.float32)
    a32 = np.asarray(anchors, dtype=np.float32).reshape(J, E)

    nc = build_graph()
    in_maps = make_in_maps(x32, a32)
    results = run_bass_kernel_spmd(nc, in_maps, core_ids=list(range(8))).results

    out = np.empty((B, J), dtype=np.float32)
    for c in range(8):
        g, h = c // RJ, c % RJ
        out[g * MB : (g + 1) * MB, h * NJ : (h + 1) * NJ] = results[c][
            "out"
        ].astype(np.float32)
    return out.reshape(B, C, A)


# revision 9
# speedup vs baseline: 1.2537x; 1.0020x over previous
"""Pairwise L2 distance kernel: x [4096,768], anchors [100,64,768] -> [4096,100,64].

Distributed over 8 TRN2 NeuronCores as a 2x4 grid: batch (4096) split in 2,
anchor index (6400) split in 4.  Each core computes a [2048,1600] output block
as sqrt(x2[b] + a2[j] - 2*x@A^T).

The x@A^T matmul runs in fp8e4m3 with DoubleRow (K=256 per pass, fp32 PSUM
accumulate); norms are computed on device (x2 from a bf16 copy of x via
ACT Square+accum, a2 via DVE square + all-(-0.5) ones-matmul broadcast).
Host does layout transforms only (transpose, dtype cast, partition packing).
"""

import sys

import numpy as np

for _p in ("/opt/trn_rl_repo", "/root/.axon_site/_ro/trn_rl_repo"):
    if _p not in sys.path:
        sys.path.append(_p)

import ml_dtypes

import concourse.bass as bass
import concourse.tile as tile
from concourse import bacc, mybir
from concourse.bass import ts
from concourse.bass_utils import run_bass_kernel_spmd

B, C, A, E = 4096, 100, 64, 768
J = C * A                 # 6400 flattened anchors
RB, RJ = 2, 4             # batch groups x anchor groups = 8 cores
MB = B // RB              # 2048 batch rows per core
NJ = J // RJ              # 1600 anchor cols per core
KT = E // 128             # 6 contraction tiles of 128
K2 = KT // 2              # 3 DoubleRow k-pair passes
MT = MB // 128            # 16 m-tiles per core
XO_Q = 4                  # xo arrives in 4 DMA slices
N_CHUNKS = [(0, 512), (512, 512), (1024, 512), (1536, 64)]
PSW = 2048                # psum tile width (4 banks), holds all chunks

FP8 = mybir.dt.float8e4
BF16 = mybir.dt.bfloat16
F32 = mybir.dt.float32
NP_FP8 = ml_dtypes.float8_e4m3
NP_BF16 = ml_dtypes.bfloat16


def pack_rows(a2d: np.ndarray) -> np.ndarray:
    """[n*128, F] -> [128, n*F]: row r=k*128+p lands at partition p, block k.
    Makes each SBUF partition's data contiguous in DRAM (one fat DMA
    descriptor per partition instead of one per 128-row block)."""
    n = a2d.shape[0] // 128
    return np.ascontiguousarray(
        a2d.reshape(n, 128, a2d.shape[1]).transpose(1, 0, 2).reshape(128, -1)
    )


def build_graph() -> bass.Bass:
    mt_q = MT // XO_Q
    nc = bacc.Bacc(None, target_bir_lowering=False, debug=False, num_devices=8)
    at_ext = nc.declare_dram_parameter("at", [128, KT * NJ], FP8, isOutput=False)
    xt_ext = nc.declare_dram_parameter("xt", [128, KT * MB], FP8, isOutput=False)
    xo_ext = nc.declare_dram_parameter("xo", [128, MT * E], BF16, isOutput=False)
    out_ext = nc.declare_dram_parameter("out", [MB, NJ], BF16, isOutput=True)

    with tile.TileContext(nc) as tc:
        with (
            tc.tile_pool(name="big", bufs=1) as big,
            tc.tile_pool(name="k2p", bufs=K2) as k2p,
            tc.tile_pool(name="xop", bufs=XO_Q) as xop,
            tc.tile_pool(name="work", bufs=3) as work,
            tc.tile_pool(name="outs", bufs=3) as outs,
            tc.tile_pool(name="psum", bufs=2, space=bass.MemorySpace.PSUM) as psp,
        ):
            # Per-k2-pair loads so the first matmuls start as soon as the
            # first 256 contraction rows have landed.
            at2, xt2 = [], []
            for q in range(K2):
                a_t = k2p.tile([128, 2, NJ], FP8, tag="at", name=f"at{q}")
                nc.sync.dma_start(
                    out=a_t,
                    in_=at_ext[:, 2 * q * NJ : 2 * (q + 1) * NJ].rearrange(
                        "p (k n) -> p k n", k=2
                    ),
                )
                at2.append(a_t)
                x_t = k2p.tile([128, 2, MB], FP8, tag="xt", name=f"xt{q}")
                nc.sync.dma_start(
                    out=x_t,
                    in_=xt_ext[:, 2 * q * MB : 2 * (q + 1) * MB].rearrange(
                        "p (k b) -> p k b", k=2
                    ),
                )
                xt2.append(x_t)
            xo_r = xo_ext[:].rearrange("p (q e) -> p q e", q=XO_Q)
            xo_s = []
            for q in range(XO_Q):
                o_t = xop.tile([128, mt_q * E], BF16, tag="xo", name=f"xo{q}")
                nc.sync.dma_start(out=o_t, in_=xo_r[:, q, :])
                xo_s.append(o_t)

            # sq_at = at*at in bf16 (feeds the a2 broadcast matmul).
            sq2 = []
            for q in range(K2):
                s_t = k2p.tile([128, 2, NJ], BF16, tag="sqat", name=f"sqat{q}")
                nc.vector.tensor_mul(s_t, at2[q], at2[q])
                sq2.append(s_t)

            neg_half = big.tile([128, 128], BF16)
            nc.vector.memset(neg_half, -0.5)
            a2b = big.tile([128, NJ], F32)  # -0.5*a2[j], same on every partition

            def emit_a2_setup():
                ps = psp.tile([128, PSW], F32, tag="ps", name="psa2")
                for n0, w in N_CHUNKS:
                    for k in range(KT):
                        nc.tensor.matmul(
                            ps[:, n0 : n0 + w],
                            neg_half,
                            sq2[k // 2][:, k % 2, n0 : n0 + w],
                            start=(k == 0), stop=(k == KT - 1),
                        )
                nc.scalar.copy(a2b, ps[:, :NJ])

            for m in range(MT):
                pts = psp.tile([128, PSW], F32, tag="ps", name=f"ps{m}")
                for q in range(K2):
                    lhsT = xt2[q][:, :, ts(m, 128)]
                    for n0, w in N_CHUNKS:
                        nc.tensor.matmul(
                            pts[:, n0 : n0 + w],
                            lhsT,
                            at2[q][:, :, n0 : n0 + w],
                            start=(q == 0), stop=(q == K2 - 1),
                            perf_mode=mybir.MatmulPerfMode.DoubleRow,
                        )
                if m == 0:
                    # Traced after m0's matmuls: PE reaches these once at has
                    # landed; the result is ready for m0's epilogue.
                    emit_a2_setup()

                sq_x = work.tile([128, E], BF16, tag="sqx")
                x2 = work.tile([128, 1], F32, tag="x2")
                xo_m = xo_s[m // mt_q][:, (m % mt_q) * E : (m % mt_q + 1) * E]
                nc.gpsimd.tensor_mul(sq_x, xo_m, xo_m)
                nc.vector.reduce_sum(x2, sq_x, axis=mybir.AxisListType.X)

                t = work.tile([128, NJ], F32, tag="t")
                nc.vector.tensor_add(t, pts[:, :NJ], a2b)
                out_t = outs.tile([128, NJ], BF16, tag="out", name=f"out{m}")
                nc.scalar.activation(
                    out_t, t, mybir.ActivationFunctionType.Sqrt,
                    bias=x2, scale=-2.0,
                )
                nc.sync.dma_start(out=out_ext[ts(m, 128), :], in_=out_t)

    nc.compile()
    return nc


def make_in_maps(x32: np.ndarray, a32: np.ndarray) -> list[dict[str, np.ndarray]]:
    xt_f8 = x32.T.astype(NP_FP8)           # [E, B]
    xo_bf = x32.astype(NP_BF16)            # [B, E]
    at_f8 = a32.T.astype(NP_FP8)           # [E, J]
    in_maps = []
    for c in range(8):
        g, h = c // RJ, c % RJ
        in_maps.append({
            "at": pack_rows(at_f8[:, h * NJ : (h + 1) * NJ]),
            "xt": pack_rows(xt_f8[:, g * MB : (g + 1) * MB]),
            "xo": pack_rows(xo_bf[g * MB : (g + 1) * MB, :]),
        })
    return in_maps


def kernel(x: np.ndarray, anchors: np.ndarray) -> np.ndarray:
    x32 = np.asarray(x, dtype=np.float32)
    a32 = np.asarray(anchors, dtype=np.float32).reshape(J, E)

    nc = build_graph()
    in_maps = make_in_maps(x32, a32)
    results = run_bass_kernel_spmd(nc, in_maps, core_ids=list(range(8))).results

    out = np.empty((B, J), dtype=np.float32)
    for c in range(8):
        g, h = c // RJ, c % RJ
        out[g * MB : (g + 1) * MB, h * NJ : (h + 1) * NJ] = results[c][
            "out"
        ].astype(np.float32)
    return out.reshape(B, C, A)
